# revision 18
# baseline (speedup 1.0000x reference)
"""Multi-head attention (RoPE + relative-position-bias) Trainium2 kernel.

Shards across 8 NeuronCores as (batch x head-quarter): core c handles
batch c//4 and heads [4*(c%4) .. 4*(c%4)+3].  Per core: Q/K/V projections
(bf16 matmuls, fp32 PSUM), RoPE, scores = qk/sqrt(dk); the Toeplitz
relative-position bias is applied multiplicatively after exp using a
host-precomputed exp(bias) staged table, fused with the softmax row-sum
in one vector tensor_tensor_reduce op.  The normalized attention matrix
(512MB output) is written with casting bf16->f32 gpsimd DMAs.  attn @ v
runs one pipeline slot behind the softmax chain so TensorE never waits
on the exp/normalize/transpose chain; the output projection is
interleaved per row-group.  Host gathers attn slabs and sums the 4
partial out-projections per batch.
"""

import os
import sys

for _p in (
    "/root/.axon_site",
    "/root/.axon_site/_ro/trn_rl_repo",
    "/root/.axon_site/_ro/pypackages",
    "/opt/trn_rl_repo",
):
    if os.path.isdir(_p) and _p not in sys.path:
        sys.path.append(_p)

import ml_dtypes
import numpy as np

import concourse.bacc as bacc
import concourse.mybir as mybir
import concourse.tile as tile
from concourse.bass_utils import run_bass_kernel_spmd

BF16 = mybir.dt.bfloat16
F16 = mybir.dt.float16
F32 = mybir.dt.float32
NBF = ml_dtypes.bfloat16

B, S, D = 2, 2048, 1024
H, DK = 16, 64
MAXL = 2048
HPC = 4          # heads per core
NCORES = 8
NIB = S // 128   # 16 row blocks
NCH = D // 128   # 8 contraction chunks
IGRP = 4         # i-blocks per attnT group (av rhs free dim = 512)
NGRP = NIB // IGRP

_PROGRAM = None


def _build_program():
    nc = bacc.Bacc(None, target_bir_lowering=False)

    qT_d = nc.dram_tensor("qT", [D, S], BF16, kind="ExternalInput")
    kT_d = nc.dram_tensor("kT", [D, S], BF16, kind="ExternalInput")
    vT_d = nc.dram_tensor("vT", [D, S], BF16, kind="ExternalInput")
    wq_d = nc.dram_tensor("wqT", [D, HPC * DK], BF16, kind="ExternalInput")
    wk_d = nc.dram_tensor("wkT", [D, HPC * DK], BF16, kind="ExternalInput")
    wv_d = nc.dram_tensor("wvT", [D, HPC * DK], BF16, kind="ExternalInput")
    wo_d = nc.dram_tensor("woT", [HPC * DK, D], BF16, kind="ExternalInput")
    stag_d = nc.dram_tensor("stag", [HPC, 128, 2 * MAXL - 1], BF16,
                            kind="ExternalInput")
    id_d = nc.dram_tensor("ident", [128, 128], BF16, kind="ExternalInput")
    cos_d = nc.dram_tensor("cos2", [128, S], BF16, kind="ExternalInput")
    sinm_d = nc.dram_tensor("sinm2", [128, S], BF16, kind="ExternalInput")

    attn_d = nc.dram_tensor("attn_o", [HPC, S, S], F32, kind="ExternalOutput")
    outp_d = nc.dram_tensor("outp_o", [S, D], F32, kind="ExternalOutput")

    with tile.TileContext(nc) as tc:
        with (
            tc.tile_pool(name="const", bufs=1) as cpool,
            tc.tile_pool(name="mid", bufs=1) as mid,
            tc.tile_pool(name="smal", bufs=8) as smal,
        ):
            ident = cpool.tile([128, 128], BF16, tag="ident")
            nc.sync.dma_start(ident[:], id_d[:])
            cos2 = cpool.tile([128, S], BF16, tag="cos2")
            nc.sync.dma_start(cos2[:], cos_d[:])
            sinm2 = cpool.tile([128, S], BF16, tag="sinm2")
            nc.sync.dma_start(sinm2[:], sinm_d[:])
            wq = cpool.tile([128, NCH, HPC * DK], BF16, tag="wq")
            nc.sync.dma_start(wq[:], wq_d.rearrange("(c p) m -> p c m", p=128))
            wk = cpool.tile([128, NCH, HPC * DK], BF16, tag="wk")
            nc.sync.dma_start(wk[:], wk_d.rearrange("(c p) m -> p c m", p=128))
            wv = cpool.tile([128, NCH, HPC * DK], BF16, tag="wv")
            nc.sync.dma_start(wv[:], wv_d.rearrange("(c p) m -> p c m", p=128))
            wo = cpool.tile([128, 2, D], BF16, tag="wo")
            nc.sync.dma_start(wo[:], wo_d.rearrange("(c p) m -> p c m", p=128))

            # live-to-end intermediates: roped q/k head-pairs, v, ctx pairs
            q2 = [mid.tile([128, S], BF16, tag=f"q2_{p}", name=f"q2_{p}")
                  for p in range(2)]
            k2 = [mid.tile([128, S], BF16, tag=f"k2_{p}", name=f"k2_{p}")
                  for p in range(2)]
            v_sb = mid.tile([128, NIB, HPC * DK], BF16, tag="v_sb")
            ctx = [mid.tile([128, S], BF16, tag=f"ctx_{p}", name=f"ctx_{p}")
                   for p in range(2)]

            # ---------------- projection + RoPE phase ----------------
            with tc.tile_pool(name="pin", bufs=1) as pin:
                qTc, kTc, vTc = [], [], []
                for c in range(NCH):
                    for lst, dram, nm in ((qTc, qT_d, "q"), (kTc, kT_d, "k"),
                                          (vTc, vT_d, "v")):
                        t = pin.tile([128, S], BF16, tag=f"{nm}T{c}",
                                     name=f"{nm}T{c}")
                        nc.sync.dma_start(
                            t[:], dram[c * 128:(c + 1) * 128, :])
                        lst.append(t)

                with (
                    tc.tile_pool(name="ppsum", bufs=2, space="PSUM") as ppsum,
                    tc.tile_pool(name="pscr", bufs=2) as pscr,
                ):
                    for w_t, src_c, dsts in ((wq, qTc, q2), (wk, kTc, k2)):
                        for p in range(2):
                            ps = ppsum.tile([128, S], F32, tag="ps")
                            for c in range(NCH):
                                for n in range(S // 512):
                                    nc.tensor.matmul(
                                        ps[:, n * 512:(n + 1) * 512],
                                        w_t[:, c, p * 128:(p + 1) * 128],
                                        src_c[c][:, n * 512:(n + 1) * 512],
                                        start=(c == 0),
                                        stop=(c == NCH - 1),
                                    )
                            # evacuate to bf16 on ACT, rope on DVE in bf16;
                            # the rotate-half partition swap is done with
                            # small SBUF->SBUF DMAs (cross-partition)
                            qb = pscr.tile([128, S], BF16, tag="qb")
                            nc.scalar.activation(
                                qb[:], ps[:],
                                mybir.ActivationFunctionType.Copy)
                            qsw = pscr.tile([128, S], BF16, tag="qsw")
                            for q in range(4):
                                srcq = (q ^ 1) * 32
                                nc.gpsimd.dma_start(
                                    qsw[q * 32:(q + 1) * 32, :],
                                    qb[srcq:srcq + 32, :])
                            t1 = pscr.tile([128, S], BF16, tag="t1")
                            t2 = pscr.tile([128, S], BF16, tag="t2")
                            nc.vector.tensor_mul(t1[:], qb[:], cos2[:])
                            nc.vector.tensor_mul(t2[:], qsw[:], sinm2[:])
                            nc.vector.tensor_add(dsts[p][:], t1[:], t2[:])

                # V projection (natural [s, 4*dk] layout)
                with tc.tile_pool(name="vpsum", bufs=2, space="PSUM") as vpsum:
                    for sb in range(NIB):
                        vps = vpsum.tile([128, HPC * DK], F32, tag="vps")
                        for c in range(NCH):
                            nc.tensor.matmul(
                                vps[:],
                                vTc[c][:, sb * 128:(sb + 1) * 128],
                                wv[:, c, :],
                                start=(c == 0),
                                stop=(c == NCH - 1),
                            )
                        nc.vector.tensor_copy(v_sb[:, sb, :], vps[:])

            # ---------------- attention + output projection ----------------
            with (
                tc.tile_pool(name="stagp", bufs=1) as stagp,
                tc.tile_pool(name="spsum", bufs=3, space="PSUM") as spsum,
                tc.tile_pool(name="cpsum", bufs=2, space="PSUM") as cpsum,
                tc.tile_pool(name="expp", bufs=3) as expp,
                tc.tile_pool(name="normp", bufs=2) as normp,
                tc.tile_pool(name="attp", bufs=2) as attp,
            ):
                stag = stagp.tile([128, HPC, 2 * MAXL - 1], BF16, tag="stag")
                nc.sync.dma_start(stag[:], stag_d.rearrange("h p m -> p h m"))

                def softmax_chain(hh, g, attnT):
                    pr, po = hh // 2, (hh % 2) * 64
                    norm_g = normp.tile([128, IGRP, S], BF16, tag="norm",
                                        name="norm_g")
                    for ibl in range(IGRP):
                        ib = g * IGRP + ibl
                        i0 = ib * 128
                        off = MAXL - 1 - i0
                        exp_t = expp.tile([128, S], BF16, tag="exp",
                                          name="exp_t")
                        sums2 = smal.tile([128, 2], F32, tag="sums2",
                                          name="sums2")
                        for half in range(2):
                            sh = spsum.tile([128, 1024], F32, tag="sh",
                                            name="sh")
                            for n in range(2):
                                j0 = half * 1024 + n * 512
                                nc.tensor.matmul(
                                    sh[:, n * 512:(n + 1) * 512],
                                    q2[pr][po:po + 64, i0:i0 + 128],
                                    k2[pr][po:po + 64, j0:j0 + 512],
                                    start=True,
                                    stop=False,
                                )
                            for n in range(2):
                                j0 = half * 1024 + n * 512
                                nc.tensor.matmul(
                                    sh[:, n * 512:(n + 1) * 512],
                                    ident[:],
                                    stag[:, hh, off + j0:off + j0 + 512],
                                    start=False,
                                    stop=True,
                                )
                            nc.scalar.activation(
                                exp_t[:, half * 1024:(half + 1) * 1024],
                                sh[:],
                                mybir.ActivationFunctionType.Exp,
                                accum_out=sums2[:, half:half + 1],
                            )
                        sums = smal.tile([128, 1], F32, tag="sums",
                                         name="sums")
                        nc.vector.tensor_add(sums[:], sums2[:, 0:1],
                                             sums2[:, 1:2])
                        recip = smal.tile([128, 1], F32, tag="recip",
                                          name="recip")
                        nc.vector.reciprocal(recip[:], sums[:])
                        nc.vector.tensor_scalar_mul(norm_g[:, ibl, :],
                                                    exp_t[:], recip[:])
                        # full-size normalized attention out (casting DMA)
                        nc.gpsimd.dma_start(
                            attn_d[hh, i0:i0 + 128, :], norm_g[:, ibl, :])
                    # one whole-group transpose for the attn @ v contraction
                    eng = nc.sync if (g * HPC + hh) % 2 == 0 else nc.scalar
                    eng.dma_start_transpose(
                        attnT[:], norm_g.rearrange("p a b -> p (a b)"))

                def issue_av(hh, g, attnT):
                    pr, po = hh // 2, (hh % 2) * 64
                    attnT_r = attnT.rearrange("p (a c) m -> p c a m", c=NIB)
                    cps = cpsum.tile([64, IGRP * 128], F32, tag="cps",
                                     name="cps")
                    for jc in range(NIB):
                        nc.tensor.matmul(
                            cps[:],
                            v_sb[:, jc, hh * DK:(hh + 1) * DK],
                            attnT_r[:, jc, :, :],
                            start=(jc == 0),
                            stop=(jc == NIB - 1),
                        )
                    nc.vector.tensor_copy(
                        ctx[pr][po:po + 64, g * 512:(g + 1) * 512], cps[:])

                from collections import deque
                pend = deque()
                for g in range(NGRP):
                    for hh in range(HPC):
                        if len(pend) >= 2:
                            p = pend.popleft()
                            issue_av(*p)
                        attnT = attp.tile([128, IGRP * NIB, 128], BF16,
                                          tag="attnT", name="attnT")
                        softmax_chain(hh, g, attnT)
                        pend.append((hh, g, attnT))
                while pend:
                    p = pend.popleft()
                    issue_av(*p)

            with (
                tc.tile_pool(name="opsum", bufs=2, space="PSUM") as opsum,
                tc.tile_pool(name="outp", bufs=2) as outp,
            ):
                for ib in range(NIB):
                    i0 = ib * 128
                    osb = outp.tile([128, D], F32, tag="osb", name="osb")
                    for n in range(2):
                        ops = opsum.tile([128, 512], F32, tag="ops",
                                         name="ops")
                        for p in range(2):
                            nc.tensor.matmul(
                                ops[:],
                                ctx[p][:, i0:i0 + 128],
                                wo[:, p, n * 512:(n + 1) * 512],
                                start=(p == 0),
                                stop=(p == 1),
                            )
                        nc.vector.tensor_copy(
                            osb[:, n * 512:(n + 1) * 512], ops[:])
                    nc.gpsimd.dma_start(outp_d[i0:i0 + 128, :], osb[:])

    nc.compile()
    return nc


def _host_inputs(query, key, value, Wq, Wk, Wv, Wo, rel_table):
    """Build the 8 per-core input maps."""
    scale = 1.0 / np.sqrt(np.float32(DK))
    # RoPE tables in [d, s] layout for a 2-head (128-partition) pack
    inv_freq = 1.0 / (10000.0 ** (np.arange(0, D, 2, dtype=np.float64) / D))
    inv_freq = inv_freq[: DK // 2]
    pos = np.arange(S, dtype=np.float64)
    fr = inv_freq[:, None] * pos[None, :]          # [32, S]
    cos1 = np.concatenate([np.cos(fr), np.cos(fr)], axis=0)   # [64, S]
    sin1 = np.concatenate([-np.sin(fr), np.sin(fr)], axis=0)  # [64, S]
    cos2 = np.concatenate([cos1, cos1], axis=0).astype(NBF)   # [128, S]
    sinm2 = np.concatenate([sin1, sin1], axis=0).astype(NBF)

    ident = np.eye(128, dtype=NBF)
    # Toeplitz staging: stag[hh, p, m] = t_h[4094 + p - m]
    idx = 4094 + np.arange(128)[:, None] - np.arange(2 * MAXL - 1)[None, :]
    idx = np.clip(idx, 0, 2 * MAXL - 2)

    in_maps = []
    for c in range(NCORES):
        b, g = c // 4, c % 4
        blk = slice(g * HPC * DK, (g + 1) * HPC * DK)
        stag = np.empty((HPC, 128, 2 * MAXL - 1), dtype=NBF)
        for hh in range(HPC):
            t = np.asarray(rel_table[:, g * HPC + hh], dtype=np.float32)
            stag[hh] = t[idx].astype(NBF)
        in_maps.append({
            "qT": np.ascontiguousarray(query[b].T).astype(NBF),
            "kT": np.ascontiguousarray(key[b].T).astype(NBF),
            "vT": np.ascontiguousarray(value[b].T).astype(NBF),
            "wqT": np.ascontiguousarray((Wq[blk] * scale).T).astype(NBF),
            "wkT": np.ascontiguousarray(Wk[blk].T).astype(NBF),
            "wvT": np.ascontiguousarray(Wv[blk].T).astype(NBF),
            "woT": np.ascontiguousarray(Wo[:, blk].T).astype(NBF),
            "stag": stag,
            "ident": ident,
            "cos2": cos2,
            "sinm2": sinm2,
        })
    return in_maps


def kernel(query, key, value, Wq, Wk, Wv, Wo, bo, rel_table, _trace=False):
    global _PROGRAM
    query = np.asarray(query, dtype=np.float32)
    key = np.asarray(key, dtype=np.float32)
    value = np.asarray(value, dtype=np.float32)
    Wq = np.asarray(Wq, dtype=np.float32)
    Wk = np.asarray(Wk, dtype=np.float32)
    Wv = np.asarray(Wv, dtype=np.float32)
    Wo = np.asarray(Wo, dtype=np.float32)
    bo = np.asarray(bo, dtype=np.float32)
    rel_table = np.asarray(rel_table, dtype=np.float32)

    if _PROGRAM is None:
        _PROGRAM = _build_program()
    nc = _PROGRAM

    in_maps = _host_inputs(query, key, value, Wq, Wk, Wv, Wo, rel_table)
    kw = {}
    if _trace:
        kw = dict(trace=True, stitch_traces=False, trace_cores=[0],
                  tmpdir=os.environ.get("KTRACE_DIR") or None)
    res = run_bass_kernel_spmd(nc, in_maps, core_ids=list(range(NCORES)), **kw)

    attn = np.empty((B, H, S, S), dtype=np.float32)
    out = np.zeros((B, S, D), dtype=np.float32)
    for c in range(NCORES):
        b, g = c // 4, c % 4
        attn[b, g * HPC:(g + 1) * HPC] = res.results[c]["attn_o"]
        out[b] += res.results[c]["outp_o"]
    out += bo[None, None, :]
    if _trace:
        kernel._last_results = res
    return out, attn


# revision 21
# speedup vs baseline: 1.0712x; 1.0712x over previous
"""Multi-head attention (RoPE + relative-position-bias) Trainium2 kernel.

Shards across 8 NeuronCores as (batch x head-quarter): core c handles
batch c//4 and heads [4*(c%4) .. 4*(c%4)+3].  Per core: Q/K/V projections
(bf16 matmuls, fp32 PSUM), RoPE, scores = qk/sqrt(dk); the Toeplitz
relative-position bias is applied multiplicatively after exp using a
host-precomputed exp(bias) staged table, fused with the softmax row-sum
in one vector tensor_tensor_reduce op.  The normalized attention matrix
(512MB output) is written with casting bf16->f32 gpsimd DMAs.  attn @ v
runs one pipeline slot behind the softmax chain so TensorE never waits
on the exp/normalize/transpose chain; the output projection is
interleaved per row-group.  Host gathers attn slabs and sums the 4
partial out-projections per batch.
"""

import os
import sys

for _p in (
    "/root/.axon_site",
    "/root/.axon_site/_ro/trn_rl_repo",
    "/root/.axon_site/_ro/pypackages",
    "/opt/trn_rl_repo",
):
    if os.path.isdir(_p) and _p not in sys.path:
        sys.path.append(_p)

import ml_dtypes
import numpy as np

import concourse.bacc as bacc
import concourse.mybir as mybir
import concourse.tile as tile
from concourse.tile_rust import add_dep_helper
from concourse.bass_utils import run_bass_kernel_spmd

BF16 = mybir.dt.bfloat16
F16 = mybir.dt.float16
F32 = mybir.dt.float32
NBF = ml_dtypes.bfloat16

B, S, D = 2, 2048, 1024
H, DK = 16, 64
MAXL = 2048
HPC = 4          # heads per core
NCORES = 8
NIB = S // 128   # 16 row blocks
NCH = D // 128   # 8 contraction chunks
IGRP = 4         # i-blocks per attnT group (av rhs free dim = 512)
NGRP = NIB // IGRP

_PROGRAM = None


def _build_program():
    nc = bacc.Bacc(None, target_bir_lowering=False)

    qT_d = nc.dram_tensor("qT", [D, S], BF16, kind="ExternalInput")
    kT_d = nc.dram_tensor("kT", [D, S], BF16, kind="ExternalInput")
    vT_d = nc.dram_tensor("vT", [D, S], BF16, kind="ExternalInput")
    wq_d = nc.dram_tensor("wqT", [D, HPC * DK], BF16, kind="ExternalInput")
    wk_d = nc.dram_tensor("wkT", [D, HPC * DK], BF16, kind="ExternalInput")
    wv_d = nc.dram_tensor("wvT", [D, HPC * DK], BF16, kind="ExternalInput")
    wo_d = nc.dram_tensor("woT", [HPC * DK, D], BF16, kind="ExternalInput")
    stag_d = nc.dram_tensor("stag", [HPC, 128, 2 * MAXL - 1], BF16,
                            kind="ExternalInput")
    id_d = nc.dram_tensor("ident", [128, 128], BF16, kind="ExternalInput")
    cos_d = nc.dram_tensor("cos2", [128, S], BF16, kind="ExternalInput")
    sinm_d = nc.dram_tensor("sinm2", [128, S], BF16, kind="ExternalInput")

    attn_d = nc.dram_tensor("attn_o", [HPC, S, S], F32, kind="ExternalOutput")
    outp_d = nc.dram_tensor("outp_o", [S, D], F32, kind="ExternalOutput")

    with tile.TileContext(nc) as tc:
        with (
            tc.tile_pool(name="const", bufs=1) as cpool,
            tc.tile_pool(name="mid", bufs=1) as mid,
            tc.tile_pool(name="smal", bufs=8) as smal,
        ):
            ident = cpool.tile([128, 128], BF16, tag="ident")
            nc.sync.dma_start(ident[:], id_d[:])
            cos2 = cpool.tile([128, S], BF16, tag="cos2")
            nc.sync.dma_start(cos2[:], cos_d[:])
            sinm2 = cpool.tile([128, S], BF16, tag="sinm2")
            nc.sync.dma_start(sinm2[:], sinm_d[:])
            wq = cpool.tile([128, NCH, HPC * DK], BF16, tag="wq")
            nc.sync.dma_start(wq[:], wq_d.rearrange("(c p) m -> p c m", p=128))
            wk = cpool.tile([128, NCH, HPC * DK], BF16, tag="wk")
            nc.sync.dma_start(wk[:], wk_d.rearrange("(c p) m -> p c m", p=128))
            wv = cpool.tile([128, NCH, HPC * DK], BF16, tag="wv")
            nc.sync.dma_start(wv[:], wv_d.rearrange("(c p) m -> p c m", p=128))
            wo = cpool.tile([128, 2, D], BF16, tag="wo")
            nc.sync.dma_start(wo[:], wo_d.rearrange("(c p) m -> p c m", p=128))

            # live-to-end intermediates: roped q/k head-pairs, v, ctx pairs
            q2 = [mid.tile([128, S], BF16, tag=f"q2_{p}", name=f"q2_{p}")
                  for p in range(2)]
            k2 = [mid.tile([128, S], BF16, tag=f"k2_{p}", name=f"k2_{p}")
                  for p in range(2)]
            v_sb = mid.tile([128, NIB, HPC * DK], BF16, tag="v_sb")
            ctx = [mid.tile([128, S], BF16, tag=f"ctx_{p}", name=f"ctx_{p}")
                   for p in range(2)]

            # ---------------- projection + RoPE phase ----------------
            with tc.tile_pool(name="pin", bufs=1) as pin:
                qTc, kTc, vTc = [], [], []
                for c in range(NCH):
                    for lst, dram, nm in ((qTc, qT_d, "q"), (kTc, kT_d, "k"),
                                          (vTc, vT_d, "v")):
                        t = pin.tile([128, S], BF16, tag=f"{nm}T{c}",
                                     name=f"{nm}T{c}")
                        nc.sync.dma_start(
                            t[:], dram[c * 128:(c + 1) * 128, :])
                        lst.append(t)

                with (
                    tc.tile_pool(name="ppsum", bufs=2, space="PSUM") as ppsum,
                    tc.tile_pool(name="pscr", bufs=2) as pscr,
                ):
                    for w_t, src_c, dsts in ((wq, qTc, q2), (wk, kTc, k2)):
                        for p in range(2):
                            ps = ppsum.tile([128, S], F32, tag="ps")
                            for c in range(NCH):
                                for n in range(S // 512):
                                    nc.tensor.matmul(
                                        ps[:, n * 512:(n + 1) * 512],
                                        w_t[:, c, p * 128:(p + 1) * 128],
                                        src_c[c][:, n * 512:(n + 1) * 512],
                                        start=(c == 0),
                                        stop=(c == NCH - 1),
                                    )
                            # evacuate to bf16 on ACT, rope on DVE in bf16;
                            # the rotate-half partition swap is done with
                            # small SBUF->SBUF DMAs (cross-partition)
                            qb = pscr.tile([128, S], BF16, tag="qb")
                            nc.scalar.activation(
                                qb[:], ps[:],
                                mybir.ActivationFunctionType.Copy)
                            qsw = pscr.tile([128, S], BF16, tag="qsw")
                            for q in range(4):
                                srcq = (q ^ 1) * 32
                                nc.gpsimd.dma_start(
                                    qsw[q * 32:(q + 1) * 32, :],
                                    qb[srcq:srcq + 32, :])
                            t1 = pscr.tile([128, S], BF16, tag="t1")
                            t2 = pscr.tile([128, S], BF16, tag="t2")
                            nc.vector.tensor_mul(t1[:], qb[:], cos2[:])
                            nc.vector.tensor_mul(t2[:], qsw[:], sinm2[:])
                            nc.vector.tensor_add(dsts[p][:], t1[:], t2[:])

                # V projection (natural [s, 4*dk] layout)
                with tc.tile_pool(name="vpsum", bufs=2, space="PSUM") as vpsum:
                    for sb in range(NIB):
                        vps = vpsum.tile([128, HPC * DK], F32, tag="vps")
                        for c in range(NCH):
                            nc.tensor.matmul(
                                vps[:],
                                vTc[c][:, sb * 128:(sb + 1) * 128],
                                wv[:, c, :],
                                start=(c == 0),
                                stop=(c == NCH - 1),
                            )
                        nc.vector.tensor_copy(v_sb[:, sb, :], vps[:])

            # ---------------- attention + output projection ----------------
            with (
                tc.tile_pool(name="stagp", bufs=1) as stagp,
                tc.tile_pool(name="spsum", bufs=3, space="PSUM") as spsum,
                tc.tile_pool(name="cpsum", bufs=1, space="PSUM") as cpsum,
                tc.tile_pool(name="opsum", bufs=1, space="PSUM") as opsum,
                tc.tile_pool(name="expp", bufs=3) as expp,
                tc.tile_pool(name="normp", bufs=2) as normp,
                tc.tile_pool(name="attp", bufs=2) as attp,
                tc.tile_pool(name="outp", bufs=2) as outp,
            ):
                stag = stagp.tile([128, HPC, 2 * MAXL - 1], BF16, tag="stag")
                nc.sync.dma_start(stag[:], stag_d.rearrange("h p m -> p h m"))

                def softmax_chain(hh, g, attnT):
                    pr, po = hh // 2, (hh % 2) * 64
                    norm_g = normp.tile([128, IGRP, S], BF16, tag="norm",
                                        name="norm_g")
                    for ibl in range(IGRP):
                        ib = g * IGRP + ibl
                        i0 = ib * 128
                        off = MAXL - 1 - i0
                        exp_t = expp.tile([128, S], BF16, tag="exp",
                                          name="exp_t")
                        sums2 = smal.tile([128, 2], F32, tag="sums2",
                                          name="sums2")
                        for half in range(2):
                            sh = spsum.tile([128, 1024], F32, tag="sh",
                                            name="sh")
                            for n in range(2):
                                j0 = half * 1024 + n * 512
                                nc.tensor.matmul(
                                    sh[:, n * 512:(n + 1) * 512],
                                    q2[pr][po:po + 64, i0:i0 + 128],
                                    k2[pr][po:po + 64, j0:j0 + 512],
                                    start=True,
                                    stop=False,
                                )
                            for n in range(2):
                                j0 = half * 1024 + n * 512
                                nc.tensor.matmul(
                                    sh[:, n * 512:(n + 1) * 512],
                                    ident[:],
                                    stag[:, hh, off + j0:off + j0 + 512],
                                    start=False,
                                    stop=True,
                                )
                            nc.scalar.activation(
                                exp_t[:, half * 1024:(half + 1) * 1024],
                                sh[:],
                                mybir.ActivationFunctionType.Exp,
                                accum_out=sums2[:, half:half + 1],
                            )
                        sums = smal.tile([128, 1], F32, tag="sums",
                                         name="sums")
                        nc.vector.tensor_add(sums[:], sums2[:, 0:1],
                                             sums2[:, 1:2])
                        recip = smal.tile([128, 1], F32, tag="recip",
                                          name="recip")
                        nc.vector.reciprocal(recip[:], sums[:])
                        nc.vector.tensor_scalar_mul(norm_g[:, ibl, :],
                                                    exp_t[:], recip[:])
                        # full-size normalized attention out (casting DMA)
                        nc.gpsimd.dma_start(
                            attn_d[hh, i0:i0 + 128, :], norm_g[:, ibl, :])
                    return norm_g

                def issue_av(hh, g, attnT, t_inst):
                    pr, po = hh // 2, (hh % 2) * 64
                    attnT_r = attnT.rearrange("p (a c) m -> p c a m", c=NIB)
                    cps = cpsum.tile([64, IGRP * 128], F32, tag="cps",
                                     name="cps")
                    for jc in range(NIB):
                        mm = nc.tensor.matmul(
                            cps[:],
                            v_sb[:, jc, hh * DK:(hh + 1) * DK],
                            attnT_r[:, jc, :, :],
                            start=(jc == 0),
                            stop=(jc == NIB - 1),
                        )
                        if t_inst is not None:
                            add_dep_helper(mm.ins, t_inst.ins,
                                           reason="av reads transposed attn")
                    nc.vector.tensor_copy(
                        ctx[pr][po:po + 64, g * 512:(g + 1) * 512], cps[:])

                def issue_outproj(g):
                    for ibl in range(IGRP):
                        ib = g * IGRP + ibl
                        i0 = ib * 128
                        osb = outp.tile([128, D], F32, tag="osb", name="osb")
                        for n in range(2):
                            ops = opsum.tile([128, 512], F32, tag="ops",
                                             name="ops")
                            for p in range(2):
                                nc.tensor.matmul(
                                    ops[:],
                                    ctx[p][:, i0:i0 + 128],
                                    wo[:, p, n * 512:(n + 1) * 512],
                                    start=(p == 0),
                                    stop=(p == 1),
                                )
                            nc.vector.tensor_copy(
                                osb[:, n * 512:(n + 1) * 512], ops[:])
                        nc.gpsimd.dma_start(outp_d[i0:i0 + 128, :], osb[:])

                def issue_transpose(hh, g, norm_g, attnT):
                    eng = nc.sync if (g * HPC + hh) % 2 == 0 else nc.scalar
                    return eng.dma_start_transpose(
                        attnT[:], norm_g.rearrange("p a b -> p (a b)"))

                from collections import deque
                pend_t = deque()   # softmax done, transpose not yet issued
                pend_av = deque()  # transpose issued, av not yet issued
                slots = [(g, hh) for g in range(NGRP) for hh in range(HPC)]

                def drain(min_t, min_av):
                    if len(pend_t) >= min_t:
                        hh0, g0, norm_g0, attnT0 = pend_t.popleft()
                        ti = issue_transpose(hh0, g0, norm_g0, attnT0)
                        pend_av.append((hh0, g0, attnT0, ti))
                    if len(pend_av) >= min_av:
                        hh1, g1, attnT1, ti1 = pend_av.popleft()
                        issue_av(hh1, g1, attnT1, ti1)
                        if hh1 == HPC - 1:
                            issue_outproj(g1)

                for g, hh in slots:
                    drain(2, 2)
                    attnT = attp.tile([128, IGRP * NIB, 128], BF16,
                                      tag="attnT", name="attnT")
                    norm_g = softmax_chain(hh, g, attnT)
                    pend_t.append((hh, g, norm_g, attnT))
                while pend_t or pend_av:
                    drain(1, 1)

    nc.compile()
    return nc


def _host_inputs(query, key, value, Wq, Wk, Wv, Wo, rel_table):
    """Build the 8 per-core input maps."""
    scale = 1.0 / np.sqrt(np.float32(DK))
    # RoPE tables in [d, s] layout for a 2-head (128-partition) pack
    inv_freq = 1.0 / (10000.0 ** (np.arange(0, D, 2, dtype=np.float64) / D))
    inv_freq = inv_freq[: DK // 2]
    pos = np.arange(S, dtype=np.float64)
    fr = inv_freq[:, None] * pos[None, :]          # [32, S]
    cos1 = np.concatenate([np.cos(fr), np.cos(fr)], axis=0)   # [64, S]
    sin1 = np.concatenate([-np.sin(fr), np.sin(fr)], axis=0)  # [64, S]
    cos2 = np.concatenate([cos1, cos1], axis=0).astype(NBF)   # [128, S]
    sinm2 = np.concatenate([sin1, sin1], axis=0).astype(NBF)

    ident = np.eye(128, dtype=NBF)
    # Toeplitz staging: stag[hh, p, m] = t_h[4094 + p - m]
    idx = 4094 + np.arange(128)[:, None] - np.arange(2 * MAXL - 1)[None, :]
    idx = np.clip(idx, 0, 2 * MAXL - 2)

    in_maps = []
    for c in range(NCORES):
        b, g = c // 4, c % 4
        blk = slice(g * HPC * DK, (g + 1) * HPC * DK)
        stag = np.empty((HPC, 128, 2 * MAXL - 1), dtype=NBF)
        for hh in range(HPC):
            t = np.asarray(rel_table[:, g * HPC + hh], dtype=np.float32)
            stag[hh] = t[idx].astype(NBF)
        in_maps.append({
            "qT": np.ascontiguousarray(query[b].T).astype(NBF),
            "kT": np.ascontiguousarray(key[b].T).astype(NBF),
            "vT": np.ascontiguousarray(value[b].T).astype(NBF),
            "wqT": np.ascontiguousarray((Wq[blk] * scale).T).astype(NBF),
            "wkT": np.ascontiguousarray(Wk[blk].T).astype(NBF),
            "wvT": np.ascontiguousarray(Wv[blk].T).astype(NBF),
            "woT": np.ascontiguousarray(Wo[:, blk].T).astype(NBF),
            "stag": stag,
            "ident": ident,
            "cos2": cos2,
            "sinm2": sinm2,
        })
    return in_maps


def kernel(query, key, value, Wq, Wk, Wv, Wo, bo, rel_table, _trace=False):
    global _PROGRAM
    query = np.asarray(query, dtype=np.float32)
    key = np.asarray(key, dtype=np.float32)
    value = np.asarray(value, dtype=np.float32)
    Wq = np.asarray(Wq, dtype=np.float32)
    Wk = np.asarray(Wk, dtype=np.float32)
    Wv = np.asarray(Wv, dtype=np.float32)
    Wo = np.asarray(Wo, dtype=np.float32)
    bo = np.asarray(bo, dtype=np.float32)
    rel_table = np.asarray(rel_table, dtype=np.float32)

    if _PROGRAM is None:
        _PROGRAM = _build_program()
    nc = _PROGRAM

    in_maps = _host_inputs(query, key, value, Wq, Wk, Wv, Wo, rel_table)
    kw = {}
    if _trace:
        kw = dict(trace=True, stitch_traces=False, trace_cores=[0],
                  tmpdir=os.environ.get("KTRACE_DIR") or None)
    res = run_bass_kernel_spmd(nc, in_maps, core_ids=list(range(NCORES)), **kw)

    attn = np.empty((B, H, S, S), dtype=np.float32)
    out = np.zeros((B, S, D), dtype=np.float32)
    for c in range(NCORES):
        b, g = c // 4, c % 4
        attn[b, g * HPC:(g + 1) * HPC] = res.results[c]["attn_o"]
        out[b] += res.results[c]["outp_o"]
    out += bo[None, None, :]
    if _trace:
        kernel._last_results = res
    return out, attn
